# revision 3
# baseline (speedup 1.0000x reference)
"""Multi-head attention (b=4, n=2048, dim=512, h=8, dh=64) on 8 trn2 cores. v2.

Sharding: core c handles batch c//2, query rows [(c%2)*1024, ...+1024).
K/V computed redundantly per batch pair. No collectives.

v2 changes over baseline:
  - out-projection contracts K=128: aot stored [128=(h_even d | h_odd d), 4, nq],
    w_out host-arranged [128, 4, dim]; 4 matmuls + bias vs 8+1.
  - input DMA prologue split into 512-col pieces across SP + Pool queues,
    ordered by first use, so the first kt matmul starts ~3us in.
  - softmax 1/l bounce batched: one 2-row DRAM write + one [128,512]
    broadcast read per head-pair (was 2+2), landing rb for both heads.
  - out-projection of chunk c deferred until after (c+1, p=0) attention is
    emitted -> the 1/l DMA latency is off the PE critical path.
  - act exp table preloaded via dummy activation during the DMA prologue.
  - output stored bf16 (host converts back to fp32).
  - optional: approx4 of every 4 key-tiles' exp computed on DVE via
    Schraudolph int16 bit-trick (offloads the Act engine bottleneck).
  - optional: dupbd scores — K=128 block-diagonal score matmuls using
    duplicated KT/QT (avoids the measured ~1.7x K=64 matmul penalty).
"""

import numpy as np
import ml_dtypes

B, N, DIM = 4, 2048, 512
H, DH, INNER = 8, 64, 512
NCORES = 8

_BUILT = {}

SCH_A = float(np.log2(np.e) * 128.0)   # bf16 Schraudolph multiplier
SCH_B = 127.0 * 128.0 - 6.0            # exponent bias + tuning constant


def build_module(dim=DIM, h=H, nq=N // 2, nk=N, compile_module=True, reps=1,
                 approx4=1, dupbd=False, gp_copy=False, stbufs=2, accbufs=4,
                 oproj_early=False):
    """approx4: 0..4 — of every 4 key-tiles, how many use DVE bit-trick exp.
    dupbd: use duplicated-KT/QT block-diagonal K=128 score matmuls.
    gp_copy: issue qt + fo copies on gpsimd instead of DVE."""
    import concourse.mybir as mybir
    import concourse.tile as tile
    from concourse import bacc

    CDT = mybir.dt.bfloat16
    FDT = mybir.dt.float32
    I16 = mybir.dt.int16
    Exp = mybir.ActivationFunctionType.Exp
    Copy = mybir.ActivationFunctionType.Copy
    Mult = mybir.AluOpType.mult
    Add = mybir.AluOpType.add

    inner = h * DH
    nkt = dim // 128          # contraction tiles for projections
    npr = h // 2              # head pairs
    nj = nk // 128            # key tiles
    nqc = nq // 512           # query chunks
    VW = DH + 1               # per-head v columns + ones column
    scale = DH ** -0.5

    nc = bacc.Bacc("TRN2", target_bir_lowering=False, debug=False,
                   num_devices=NCORES)

    xt_d = nc.declare_dram_parameter("xt", [dim, nq], CDT, isOutput=False)
    xpt_d = nc.declare_dram_parameter("xpt", [dim, nk], CDT, isOutput=False)
    wq_d = nc.declare_dram_parameter("wq", [dim, inner], CDT, isOutput=False)
    wkv_d = nc.declare_dram_parameter("wkv", [dim, 2 * inner], CDT,
                                      isOutput=False)
    # w_out pre-arranged on host: wout3[p, s, :] = w_out[s*128+p, :]
    wout_d = nc.declare_dram_parameter("wout", [128, inner // 128, dim], CDT,
                                       isOutput=False)
    bout_d = nc.declare_dram_parameter("bout", [1, dim], CDT, isOutput=False)
    out_d = nc.declare_dram_parameter("out", [nq, dim], CDT, isOutput=True)
    rsc_d = nc.dram_tensor("rscratch", [2 * nqc * npr, 512], FDT)

    import contextlib
    with tile.TileContext(nc) as tc, contextlib.ExitStack() as stack:
        consts = stack.enter_context(tc.tile_pool(name="consts", bufs=1))
        acts = stack.enter_context(tc.tile_pool(name="acts", bufs=1))

        # ---- weights / constants ----------------------------------------
        wq_sb = consts.tile([128, nkt, inner], CDT)
        wkv_sb = consts.tile([128, nkt, 2 * inner], CDT)
        wout_sb = consts.tile([128, inner // 128, dim], CDT)
        bout_sb = consts.tile([1, dim], CDT)
        ones_sb = consts.tile([1, 128], CDT)
        warm_sb = consts.tile([1, 16], FDT)

        # act exp table preload in the shadow of the DMA prologue
        nc.vector.memset(warm_sb[:, :], 0.0)
        nc.scalar.activation(out=warm_sb[:, :], in_=warm_sb[:, :], func=Exp,
                             scale=1.0)
        nc.vector.memset(ones_sb[:, :], 1.0)

        # ---- activations -------------------------------------------------
        xt_sb = acts.tile([128, nkt, nq], CDT)
        xpt_sb = acts.tile([128, nkt, nk], CDT)

        wkv_r = wkv_d.ap().rearrange("(t p) o -> p t o", p=128)
        wq_r = wq_d.ap().rearrange("(t p) o -> p t o", p=128)
        xpt_r = xpt_d.ap().rearrange("(t p) n -> p t n", p=128)
        xt_r = xt_d.ap().rearrange("(t p) n -> p t n", p=128)

        # prologue DMAs, ordered by first use.
        # SP queue: wkv K-half, wq, wkv V-half, wout, bout.
        # Pool queue: xpt chunk 0, xt chunk 0, remaining xpt/xt chunks.
        for k in range(nkt):
            nc.sync.dma_start(out=wkv_sb[:, k, 0:inner],
                              in_=wkv_r[:, k, 0:inner])
            nc.gpsimd.dma_start(out=xpt_sb[:, k, 0:512],
                                in_=xpt_r[:, k, 0:512])
        for k in range(nkt):
            nc.sync.dma_start(out=wq_sb[:, k, :], in_=wq_r[:, k, :])
            nc.gpsimd.dma_start(out=xt_sb[:, k, 0:512],
                                in_=xt_r[:, k, 0:512])
        for c in range(1, nk // 512):
            for k in range(nkt):
                nc.gpsimd.dma_start(
                    out=xpt_sb[:, k, c * 512:(c + 1) * 512],
                    in_=xpt_r[:, k, c * 512:(c + 1) * 512])
            if c < nq // 512:
                for k in range(nkt):
                    nc.gpsimd.dma_start(
                        out=xt_sb[:, k, c * 512:(c + 1) * 512],
                        in_=xt_r[:, k, c * 512:(c + 1) * 512])
        for k in range(nkt):
            nc.sync.dma_start(out=wkv_sb[:, k, inner:2 * inner],
                              in_=wkv_r[:, k, inner:2 * inner])
        nc.sync.dma_start(out=wout_sb[:, :, :], in_=wout_d.ap())
        nc.sync.dma_start(out=bout_sb[:, :], in_=bout_d.ap())

        v_sb = acts.tile([128, nj, h * VW], CDT)  # [key-tile, h*(dh+1)]
        aot_sb = acts.tile([128, inner // 128, nq], CDT)
        if not dupbd:
            qt_sb = acts.tile([128, npr, nq], CDT)    # [(2h d), nq]
            kt_sb = acts.tile([128, npr, nk], CDT)    # [(2h d), nk]
        else:
            # block-diag KT per head: [128, h, nj, 128]
            ktb_sb = acts.tile([128, h, nj, 128], CDT)
            qtd_sb = acts.tile([128, h, nq], CDT)  # QT duplicated both halves
            nc.vector.memset(ktb_sb[0:64, :, :, 64:128], 0.0)
            nc.vector.memset(ktb_sb[64:128, :, :, 0:64], 0.0)

        for hh in range(h):  # ones columns of V
            nc.vector.memset(v_sb[:, :, hh * VW + DH:hh * VW + DH + 1], 1.0)

        cp_q = nc.gpsimd if gp_copy else nc.vector

        for _rep in range(reps):
            # ---- projections: kt/qt interleaved, v after slice 0 ---------
            kw = 512
            order = []
            for s in range(npr):
                if s == 0:
                    # interleave kt/qt so the PE has work while the xpt
                    # chunk DMAs stream in
                    order += [("kt", 0, 0), ("qt", 0, 0), ("kt", 0, 1),
                              ("qt", 0, 1), ("kt", 0, 2), ("kt", 0, 3)]
                    for j in range(nj):
                        order.append(("v", j, 0))
                else:
                    for c in range(nk // kw):
                        order.append(("kt", s, c))
                    for c in range(nqc):
                        order.append(("qt", s, c))
            proj_scope = tc.tile_pool(name="proj_ps", bufs=4, space="PSUM")
            proj_ps = proj_scope.__enter__()
            for kind, a, c in order:
                ps = proj_ps.tile([128, 512], FDT, tag="mm")
                if kind == "kt":
                    for k in range(nkt):
                        nc.tensor.matmul(
                            ps[:, 0:kw],
                            lhsT=wkv_sb[:, k, a * 128:(a + 1) * 128],
                            rhs=xpt_sb[:, k, c * kw:(c + 1) * kw],
                            start=(k == 0), stop=(k == nkt - 1))
                    if not dupbd:
                        nc.vector.tensor_copy(
                            out=kt_sb[:, a, c * kw:(c + 1) * kw],
                            in_=ps[:, 0:kw])
                    else:
                        # block-diag per head: tile (h, j) holds subA (keys
                        # j*128..+64) at rows 0:64/cols 0:64 and subB at
                        # rows 64:128/cols 64:128.
                        h0, h1 = 2 * a, 2 * a + 1
                        j0 = (c * kw) // 128
                        nj_c = kw // 128
                        pr = ps[:, 0:kw].rearrange(
                            "p (a t x) -> p a t x", t=2, x=64)
                        nc.vector.tensor_copy(
                            out=ktb_sb[0:64, h0, j0:j0 + nj_c, 0:64],
                            in_=pr[0:64, :, 0, :])
                        nc.vector.tensor_copy(
                            out=ktb_sb[64:128, h0, j0:j0 + nj_c, 64:128],
                            in_=pr[0:64, :, 1, :])
                        nc.vector.tensor_copy(
                            out=ktb_sb[0:64, h1, j0:j0 + nj_c, 0:64],
                            in_=pr[64:128, :, 0, :])
                        nc.vector.tensor_copy(
                            out=ktb_sb[64:128, h1, j0:j0 + nj_c, 64:128],
                            in_=pr[64:128, :, 1, :])
                elif kind == "qt":
                    for k in range(nkt):
                        nc.tensor.matmul(
                            ps[:, :], lhsT=wq_sb[:, k, a * 128:(a + 1) * 128],
                            rhs=xt_sb[:, k, c * 512:(c + 1) * 512],
                            start=(k == 0), stop=(k == nkt - 1))
                    if not dupbd:
                        cp_q.tensor_copy(
                            out=qt_sb[:, a, c * 512:(c + 1) * 512],
                            in_=ps[:, :])
                    else:
                        h0, h1 = 2 * a, 2 * a + 1
                        sl = slice(c * 512, (c + 1) * 512)
                        nc.vector.tensor_copy(out=qtd_sb[0:64, h0, sl],
                                              in_=ps[0:64, :])
                        nc.vector.tensor_copy(out=qtd_sb[64:128, h0, sl],
                                              in_=ps[0:64, :])
                        cp_q.tensor_copy(out=qtd_sb[0:64, h1, sl],
                                         in_=ps[64:128, :])
                        cp_q.tensor_copy(out=qtd_sb[64:128, h1, sl],
                                         in_=ps[64:128, :])
                else:  # v
                    for k in range(nkt):
                        nc.tensor.matmul(
                            ps[:, 0:inner],
                            lhsT=xpt_sb[:, k, a * 128:(a + 1) * 128],
                            rhs=wkv_sb[:, k, inner:2 * inner],
                            start=(k == 0), stop=(k == nkt - 1))
                    nc.vector.tensor_copy(
                        out=v_sb[:, a, :].rearrange(
                            "p (g x) -> p g x", x=VW)[:, :, 0:DH],
                        in_=ps[:, 0:inner].rearrange("p (g x) -> p g x", x=DH))
            proj_scope.__exit__(None, None, None)

            # ---- attention ----------------------------------------------
            attn_stack = contextlib.ExitStack()
            st_ps = attn_stack.enter_context(
                tc.tile_pool(name="st_ps", bufs=stbufs, space="PSUM"))
            acc_ps = attn_stack.enter_context(
                tc.tile_pool(name="acc_ps", bufs=accbufs, space="PSUM"))
            pt_pool = attn_stack.enter_context(tc.tile_pool(name="pt", bufs=2))
            lr_pool = attn_stack.enter_context(tc.tile_pool(name="lr", bufs=3))

            def scores(c, p):
                h0, h1 = 2 * p, 2 * p + 1
                pt = pt_pool.tile([128, nj, 1024], CDT, tag="pt",
                                  name="pt_t")
                pt16 = pt.bitcast(I16)
                for j in range(nj):
                    st = st_ps.tile([128, 1024], FDT, tag="st", name="st_t")
                    if dupbd:
                        nc.tensor.matmul(
                            st[:, 0:512],
                            lhsT=ktb_sb[:, h0, j, :],
                            rhs=qtd_sb[:, h0, c * 512:(c + 1) * 512],
                            start=True, stop=True)
                        nc.tensor.matmul(
                            st[:, 512:1024],
                            lhsT=ktb_sb[:, h1, j, :],
                            rhs=qtd_sb[:, h1, c * 512:(c + 1) * 512],
                            start=True, stop=True)
                    else:
                        nc.tensor.matmul(
                            st[:, 0:512],
                            lhsT=kt_sb[0:64, p, j * 128:(j + 1) * 128],
                            rhs=qt_sb[0:64, p, c * 512:(c + 1) * 512],
                            start=True, stop=True)
                        nc.tensor.matmul(
                            st[:, 512:1024],
                            lhsT=kt_sb[64:128, p, j * 128:(j + 1) * 128],
                            rhs=qt_sb[64:128, p, c * 512:(c + 1) * 512],
                            start=True, stop=True)
                    if j % 4 < approx4:
                        nc.vector.tensor_scalar(
                            out=pt16[:, j, :], in0=st[:, :],
                            scalar1=scale * SCH_A, scalar2=SCH_B,
                            op0=Mult, op1=Add)
                    else:
                        nc.scalar.activation(out=pt[:, j, :], in_=st[:, :],
                                             func=Exp, scale=scale)
                return pt

            def pv(c, p, pt):
                h0, h1 = 2 * p, 2 * p + 1
                pv0 = acc_ps.tile([128, 512], FDT, tag="acc", name="pv0_t")
                pv1 = acc_ps.tile([128, 512], FDT, tag="acc", name="pv1_t")
                for j in range(nj):
                    nc.tensor.matmul(
                        pv0[0:VW, :], lhsT=v_sb[:, j, h0 * VW:(h0 + 1) * VW],
                        rhs=pt[:, j, 0:512],
                        start=(j == 0), stop=(j == nj - 1))
                    nc.tensor.matmul(
                        pv1[0:VW, :], lhsT=v_sb[:, j, h1 * VW:(h1 + 1) * VW],
                        rhs=pt[:, j, 512:1024],
                        start=(j == 0), stop=(j == nj - 1))
                return pv0, pv1

            def norm(c, p, pv0, pv1):
                # 1/l broadcast + attention-output normalization into aot
                if c == nqc - 1 and p == npr - 1:
                    # last pair: the DRAM-bounce latency would sit on the
                    # critical tail, so broadcast 1/l via K=1 PE matmuls
                    # (bf16 reciprocals - matmul dtypes must match).
                    ra = lr_pool.tile([1, 512], CDT, tag="ra16", name="ra_t")
                    rc = lr_pool.tile([1, 512], CDT, tag="rc16", name="rc_t")
                    with nc.allow_low_precision(reason="1/l bcast in bf16"):
                        nc.vector.reciprocal(out=ra[:, :],
                                             in_=pv0[DH:DH + 1, :])
                        nc.vector.reciprocal(out=rc[:, :],
                                             in_=pv1[DH:DH + 1, :])
                    rbp = acc_ps.tile([128, 512], FDT, tag="acc",
                                      name="rbp_t")
                    nc.tensor.matmul(rbp[0:64, :], lhsT=ones_sb[:, 0:64],
                                     rhs=ra[:, :], start=True, stop=True)
                    nc.tensor.matmul(rbp[64:128, :], lhsT=ones_sb[:, 0:64],
                                     rhs=rc[:, :], start=True, stop=True)
                    # tensor ops may read only one PSUM operand - stage the
                    # broadcast rows in SBUF via the (idle) Act engine
                    rbs = lr_pool.tile([128, 512], FDT, tag="rb", name="rbs_t")
                    nc.scalar.activation(out=rbs[:, :], in_=rbp[:, :],
                                         func=Copy, scale=1.0)
                    rb0, rb1 = rbs[0:64, :], rbs[64:128, :]
                else:
                    ra = lr_pool.tile([1, 512], FDT, tag="ra", name="ra_t")
                    rc = lr_pool.tile([1, 512], FDT, tag="rc", name="rc_t")
                    nc.vector.reciprocal(out=ra[:, :], in_=pv0[DH:DH + 1, :])
                    nc.vector.reciprocal(out=rc[:, :], in_=pv1[DH:DH + 1, :])
                    idx = (c * npr + p) * 2
                    nc.sync.dma_start(out=rsc_d.ap()[idx:idx + 1, :],
                                      in_=ra[:, :])
                    nc.sync.dma_start(out=rsc_d.ap()[idx + 1:idx + 2, :],
                                      in_=rc[:, :])
                    rb = lr_pool.tile([128, 512], FDT, tag="rb", name="rb_t")
                    src = rsc_d.ap()[idx:idx + 2, :].rearrange(
                        "r (o f) -> r o f", o=1).to_broadcast([2, DH, 512])
                    nc.gpsimd.dma_start(out=rb[:, :], in_=src)
                    rb0, rb1 = rb[0:64, :], rb[64:128, :]
                nc.vector.tensor_mul(
                    aot_sb[0:64, p, c * 512:(c + 1) * 512],
                    pv0[0:DH, :], rb0)
                nc.vector.tensor_mul(
                    aot_sb[64:128, p, c * 512:(c + 1) * 512],
                    pv1[0:DH, :], rb1)

            def outproj(c):
                for t in range(4 * c, 4 * c + 4):
                    f = acc_ps.tile([128, 512], FDT, tag="acc", name="f_t")
                    for s in range(inner // 128):
                        nc.tensor.matmul(
                            f[:, 0:dim],
                            lhsT=aot_sb[:, s, t * 128:(t + 1) * 128],
                            rhs=wout_sb[:, s, :],
                            start=(s == 0), stop=False)
                    nc.tensor.matmul(f[:, 0:dim], lhsT=ones_sb[:, :],
                                     rhs=bout_sb[:, :], start=False, stop=True)
                    fo = lr_pool.tile([128, dim], CDT, tag="fo", name="fo_t")
                    nc.scalar.activation(out=fo[:, :], in_=f[:, 0:dim],
                                         func=Copy, scale=1.0)
                    nc.sync.dma_start(
                        out=out_d.ap()[t * 128:(t + 1) * 128, :], in_=fo[:, :])

            pairs = [(c, p) for c in range(nqc) for p in range(npr)]
            prev = None
            for cp in pairs:
                pt = scores(*cp)
                if prev is not None:
                    (pc, pp), (ppv0, ppv1) = prev
                    norm(pc, pp, ppv0, ppv1)
                if oproj_early and prev is not None and prev[0][1] == npr - 1:
                    outproj(prev[0][0])
                pvs = pv(*cp, pt)
                if not oproj_early and prev is not None and prev[0][1] == npr - 1:
                    outproj(prev[0][0])
                prev = (cp, pvs)
            (pc, pp), (ppv0, ppv1) = prev
            norm(pc, pp, ppv0, ppv1)
            outproj(nqc - 1)
            attn_stack.close()

    if compile_module:
        nc.compile()
    return nc


def host_inputs(x, x_prev, w_q, w_kv, w_out, b_out, ncores=NCORES):
    bf16 = ml_dtypes.bfloat16
    b, n, dim = x.shape
    inner = w_q.shape[1]
    nq = (b * n) // ncores
    halves = ncores // b
    wq = np.ascontiguousarray(w_q).astype(bf16)
    wkv = np.ascontiguousarray(w_kv).astype(bf16)
    wout = np.ascontiguousarray(
        w_out.reshape(inner // 128, 128, dim).transpose(1, 0, 2)).astype(bf16)
    bout = np.ascontiguousarray(b_out).reshape(1, dim).astype(bf16)
    in_maps = []
    for c in range(ncores):
        bb, half = c // halves, c % halves
        xt = np.ascontiguousarray(
            x[bb, half * nq:(half + 1) * nq, :].T).astype(bf16)
        xpt = np.ascontiguousarray(x_prev[bb].T).astype(bf16)
        in_maps.append(dict(xt=xt, xpt=xpt, wq=wq, wkv=wkv, wout=wout,
                            bout=bout))
    return in_maps


def _get_module(**kw):
    key = tuple(sorted(kw.items()))
    if key not in _BUILT:
        _BUILT[key] = build_module(**kw)
    return _BUILT[key]


def kernel(x, x_prev, w_q, w_kv, w_out, b_out):
    from concourse.bass_utils import run_bass_kernel_spmd

    nc = _get_module()
    in_maps = host_inputs(x, x_prev, w_q, w_kv, w_out, b_out)
    res = run_bass_kernel_spmd(nc, in_maps, core_ids=list(range(NCORES)))

    nq = N // 2
    out = np.empty((B, N, DIM), np.float32)
    for c in range(NCORES):
        b, half = c // 2, c % 2
        out[b, half * nq:(half + 1) * nq, :] = \
            res.results[c]["out"].astype(np.float32)
    return out


# revision 5
# speedup vs baseline: 1.0316x; 1.0316x over previous
"""Multi-head attention (b=4, n=2048, dim=512, h=8, dh=64) on 8 trn2 cores. v2.

Sharding: core c handles batch c//2, query rows [(c%2)*1024, ...+1024).
K/V computed redundantly per batch pair. No collectives.

v2 changes over baseline:
  - out-projection contracts K=128: aot stored [128=(h_even d | h_odd d), 4, nq],
    w_out host-arranged [128, 4, dim]; 4 matmuls + bias vs 8+1.
  - input DMA prologue split into 512-col pieces across SP + Pool queues,
    ordered by first use, so the first kt matmul starts ~3us in.
  - softmax 1/l bounce batched: one 2-row DRAM write + one [128,512]
    broadcast read per head-pair (was 2+2), landing rb for both heads.
  - out-projection of chunk c deferred until after (c+1, p=0) attention is
    emitted -> the 1/l DMA latency is off the PE critical path.
  - act exp table preloaded via dummy activation during the DMA prologue.
  - output stored bf16 (host converts back to fp32).
  - 6 of every 16 key-tiles' exp computed on DVE via a Schraudolph int16
    bit-trick (bf16 bits = round(s*log2e*128 + 127*128-6)) — offloads the
    Act engine, the real-HW bottleneck; softmax ratio cancels most of the
    ~3%% sawtooth error (total rel err 0.010 vs the 2e-2 gate).
  - optional: dupbd scores — K=128 block-diagonal score matmuls using
    duplicated KT/QT (avoids the measured ~1.7x K=64 matmul penalty).
"""

import numpy as np
import ml_dtypes

B, N, DIM = 4, 2048, 512
H, DH, INNER = 8, 64, 512
NCORES = 8

_BUILT = {}

SCH_A = float(np.log2(np.e) * 128.0)   # bf16 Schraudolph multiplier
SCH_B = 127.0 * 128.0 - 6.0            # exponent bias + tuning constant


def build_module(dim=DIM, h=H, nq=N // 2, nk=N, compile_module=True, reps=1,
                 approx16=6, dupbd=False, gp_copy=False, stbufs=2, accbufs=4,
                 oproj_early=False):
    """approx16: of every 16 key-tiles, how many use DVE bit-trick exp.
    dupbd: use duplicated-KT/QT block-diagonal K=128 score matmuls.
    gp_copy: issue qt + fo copies on gpsimd instead of DVE."""
    import concourse.mybir as mybir
    import concourse.tile as tile
    from concourse import bacc

    CDT = mybir.dt.bfloat16
    FDT = mybir.dt.float32
    I16 = mybir.dt.int16
    Exp = mybir.ActivationFunctionType.Exp
    Copy = mybir.ActivationFunctionType.Copy
    Mult = mybir.AluOpType.mult
    Add = mybir.AluOpType.add

    inner = h * DH
    nkt = dim // 128          # contraction tiles for projections
    npr = h // 2              # head pairs
    nj = nk // 128            # key tiles
    nqc = nq // 512           # query chunks
    VW = DH + 1               # per-head v columns + ones column
    scale = DH ** -0.5
    adve_sel = {0: (), 2: (0,), 4: (0, 4), 6: (0, 3, 5),
                8: (0, 2, 4, 6)}[approx16]

    nc = bacc.Bacc("TRN2", target_bir_lowering=False, debug=False,
                   num_devices=NCORES)

    xt_d = nc.declare_dram_parameter("xt", [dim, nq], CDT, isOutput=False)
    xpt_d = nc.declare_dram_parameter("xpt", [dim, nk], CDT, isOutput=False)
    wq_d = nc.declare_dram_parameter("wq", [dim, inner], CDT, isOutput=False)
    wkv_d = nc.declare_dram_parameter("wkv", [dim, 2 * inner], CDT,
                                      isOutput=False)
    # w_out pre-arranged on host: wout3[p, s, :] = w_out[s*128+p, :]
    wout_d = nc.declare_dram_parameter("wout", [128, inner // 128, dim], CDT,
                                       isOutput=False)
    bout_d = nc.declare_dram_parameter("bout", [1, dim], CDT, isOutput=False)
    out_d = nc.declare_dram_parameter("out", [nq, dim], CDT, isOutput=True)
    rsc_d = nc.dram_tensor("rscratch", [2 * nqc * npr, 512], FDT)

    import contextlib
    with tile.TileContext(nc) as tc, contextlib.ExitStack() as stack:
        consts = stack.enter_context(tc.tile_pool(name="consts", bufs=1))
        acts = stack.enter_context(tc.tile_pool(name="acts", bufs=1))

        # ---- weights / constants ----------------------------------------
        wq_sb = consts.tile([128, nkt, inner], CDT)
        wkv_sb = consts.tile([128, nkt, 2 * inner], CDT)
        wout_sb = consts.tile([128, inner // 128, dim], CDT)
        bout_sb = consts.tile([1, dim], CDT)
        ones_sb = consts.tile([1, 128], CDT)
        warm_sb = consts.tile([1, 16], FDT)

        # act exp table preload in the shadow of the DMA prologue
        nc.vector.memset(warm_sb[:, :], 0.0)
        nc.scalar.activation(out=warm_sb[:, :], in_=warm_sb[:, :], func=Exp,
                             scale=1.0)
        nc.vector.memset(ones_sb[:, :], 1.0)

        # ---- activations -------------------------------------------------
        xt_sb = acts.tile([128, nkt, nq], CDT)
        xpt_sb = acts.tile([128, nkt, nk], CDT)

        wkv_r = wkv_d.ap().rearrange("(t p) o -> p t o", p=128)
        wq_r = wq_d.ap().rearrange("(t p) o -> p t o", p=128)
        xpt_r = xpt_d.ap().rearrange("(t p) n -> p t n", p=128)
        xt_r = xt_d.ap().rearrange("(t p) n -> p t n", p=128)

        # prologue DMAs, ordered by first use.
        # SP queue: wkv K-half, wq, wkv V-half, wout, bout.
        # Pool queue: xpt chunk 0, xt chunk 0, remaining xpt/xt chunks.
        for k in range(nkt):
            nc.sync.dma_start(out=wkv_sb[:, k, 0:inner],
                              in_=wkv_r[:, k, 0:inner])
            nc.gpsimd.dma_start(out=xpt_sb[:, k, 0:512],
                                in_=xpt_r[:, k, 0:512])
        for k in range(nkt):
            nc.sync.dma_start(out=wq_sb[:, k, :], in_=wq_r[:, k, :])
            nc.gpsimd.dma_start(out=xt_sb[:, k, 0:512],
                                in_=xt_r[:, k, 0:512])
        for c in range(1, nk // 512):
            for k in range(nkt):
                nc.gpsimd.dma_start(
                    out=xpt_sb[:, k, c * 512:(c + 1) * 512],
                    in_=xpt_r[:, k, c * 512:(c + 1) * 512])
            if c < nq // 512:
                for k in range(nkt):
                    nc.gpsimd.dma_start(
                        out=xt_sb[:, k, c * 512:(c + 1) * 512],
                        in_=xt_r[:, k, c * 512:(c + 1) * 512])
        for k in range(nkt):
            nc.sync.dma_start(out=wkv_sb[:, k, inner:2 * inner],
                              in_=wkv_r[:, k, inner:2 * inner])
        nc.sync.dma_start(out=wout_sb[:, :, :], in_=wout_d.ap())
        nc.sync.dma_start(out=bout_sb[:, :], in_=bout_d.ap())

        v_sb = acts.tile([128, nj, h * VW], CDT)  # [key-tile, h*(dh+1)]
        aot_sb = acts.tile([128, inner // 128, nq], CDT)
        if not dupbd:
            qt_sb = acts.tile([128, npr, nq], CDT)    # [(2h d), nq]
            kt_sb = acts.tile([128, npr, nk], CDT)    # [(2h d), nk]
        else:
            # block-diag KT per head: [128, h, nj, 128]
            ktb_sb = acts.tile([128, h, nj, 128], CDT)
            qtd_sb = acts.tile([128, h, nq], CDT)  # QT duplicated both halves
            nc.vector.memset(ktb_sb[0:64, :, :, 64:128], 0.0)
            nc.vector.memset(ktb_sb[64:128, :, :, 0:64], 0.0)

        for hh in range(h):  # ones columns of V
            nc.vector.memset(v_sb[:, :, hh * VW + DH:hh * VW + DH + 1], 1.0)

        cp_q = nc.gpsimd if gp_copy else nc.vector

        for _rep in range(reps):
            # ---- projections: kt/qt interleaved, v after slice 0 ---------
            kw = 512
            order = []
            for s in range(npr):
                if s == 0:
                    # interleave kt/qt so the PE has work while the xpt
                    # chunk DMAs stream in
                    order += [("kt", 0, 0), ("qt", 0, 0), ("kt", 0, 1),
                              ("qt", 0, 1), ("kt", 0, 2), ("kt", 0, 3)]
                    for j in range(nj):
                        order.append(("v", j, 0))
                else:
                    for c in range(nk // kw):
                        order.append(("kt", s, c))
                    for c in range(nqc):
                        order.append(("qt", s, c))
            proj_scope = tc.tile_pool(name="proj_ps", bufs=4, space="PSUM")
            proj_ps = proj_scope.__enter__()
            for kind, a, c in order:
                ps = proj_ps.tile([128, 512], FDT, tag="mm")
                if kind == "kt":
                    for k in range(nkt):
                        nc.tensor.matmul(
                            ps[:, 0:kw],
                            lhsT=wkv_sb[:, k, a * 128:(a + 1) * 128],
                            rhs=xpt_sb[:, k, c * kw:(c + 1) * kw],
                            start=(k == 0), stop=(k == nkt - 1))
                    if not dupbd:
                        nc.vector.tensor_copy(
                            out=kt_sb[:, a, c * kw:(c + 1) * kw],
                            in_=ps[:, 0:kw])
                    else:
                        # block-diag per head: tile (h, j) holds subA (keys
                        # j*128..+64) at rows 0:64/cols 0:64 and subB at
                        # rows 64:128/cols 64:128.
                        h0, h1 = 2 * a, 2 * a + 1
                        j0 = (c * kw) // 128
                        nj_c = kw // 128
                        pr = ps[:, 0:kw].rearrange(
                            "p (a t x) -> p a t x", t=2, x=64)
                        nc.vector.tensor_copy(
                            out=ktb_sb[0:64, h0, j0:j0 + nj_c, 0:64],
                            in_=pr[0:64, :, 0, :])
                        nc.vector.tensor_copy(
                            out=ktb_sb[64:128, h0, j0:j0 + nj_c, 64:128],
                            in_=pr[0:64, :, 1, :])
                        nc.vector.tensor_copy(
                            out=ktb_sb[0:64, h1, j0:j0 + nj_c, 0:64],
                            in_=pr[64:128, :, 0, :])
                        nc.vector.tensor_copy(
                            out=ktb_sb[64:128, h1, j0:j0 + nj_c, 64:128],
                            in_=pr[64:128, :, 1, :])
                elif kind == "qt":
                    for k in range(nkt):
                        nc.tensor.matmul(
                            ps[:, :], lhsT=wq_sb[:, k, a * 128:(a + 1) * 128],
                            rhs=xt_sb[:, k, c * 512:(c + 1) * 512],
                            start=(k == 0), stop=(k == nkt - 1))
                    if not dupbd:
                        cp_q.tensor_copy(
                            out=qt_sb[:, a, c * 512:(c + 1) * 512],
                            in_=ps[:, :])
                    else:
                        h0, h1 = 2 * a, 2 * a + 1
                        sl = slice(c * 512, (c + 1) * 512)
                        nc.vector.tensor_copy(out=qtd_sb[0:64, h0, sl],
                                              in_=ps[0:64, :])
                        nc.vector.tensor_copy(out=qtd_sb[64:128, h0, sl],
                                              in_=ps[0:64, :])
                        cp_q.tensor_copy(out=qtd_sb[0:64, h1, sl],
                                         in_=ps[64:128, :])
                        cp_q.tensor_copy(out=qtd_sb[64:128, h1, sl],
                                         in_=ps[64:128, :])
                else:  # v
                    for k in range(nkt):
                        nc.tensor.matmul(
                            ps[:, 0:inner],
                            lhsT=xpt_sb[:, k, a * 128:(a + 1) * 128],
                            rhs=wkv_sb[:, k, inner:2 * inner],
                            start=(k == 0), stop=(k == nkt - 1))
                    nc.vector.tensor_copy(
                        out=v_sb[:, a, :].rearrange(
                            "p (g x) -> p g x", x=VW)[:, :, 0:DH],
                        in_=ps[:, 0:inner].rearrange("p (g x) -> p g x", x=DH))
            proj_scope.__exit__(None, None, None)

            # ---- attention ----------------------------------------------
            attn_stack = contextlib.ExitStack()
            st_ps = attn_stack.enter_context(
                tc.tile_pool(name="st_ps", bufs=stbufs, space="PSUM"))
            acc_ps = attn_stack.enter_context(
                tc.tile_pool(name="acc_ps", bufs=accbufs, space="PSUM"))
            pt_pool = attn_stack.enter_context(tc.tile_pool(name="pt", bufs=2))
            lr_pool = attn_stack.enter_context(tc.tile_pool(name="lr", bufs=3))

            def scores(c, p):
                h0, h1 = 2 * p, 2 * p + 1
                pt = pt_pool.tile([128, nj, 1024], CDT, tag="pt",
                                  name="pt_t")
                pt16 = pt.bitcast(I16)
                for j in range(nj):
                    st = st_ps.tile([128, 1024], FDT, tag="st", name="st_t")
                    if dupbd:
                        nc.tensor.matmul(
                            st[:, 0:512],
                            lhsT=ktb_sb[:, h0, j, :],
                            rhs=qtd_sb[:, h0, c * 512:(c + 1) * 512],
                            start=True, stop=True)
                        nc.tensor.matmul(
                            st[:, 512:1024],
                            lhsT=ktb_sb[:, h1, j, :],
                            rhs=qtd_sb[:, h1, c * 512:(c + 1) * 512],
                            start=True, stop=True)
                    else:
                        nc.tensor.matmul(
                            st[:, 0:512],
                            lhsT=kt_sb[0:64, p, j * 128:(j + 1) * 128],
                            rhs=qt_sb[0:64, p, c * 512:(c + 1) * 512],
                            start=True, stop=True)
                        nc.tensor.matmul(
                            st[:, 512:1024],
                            lhsT=kt_sb[64:128, p, j * 128:(j + 1) * 128],
                            rhs=qt_sb[64:128, p, c * 512:(c + 1) * 512],
                            start=True, stop=True)
                    if j % 8 in adve_sel:
                        nc.vector.tensor_scalar(
                            out=pt16[:, j, :], in0=st[:, :],
                            scalar1=scale * SCH_A, scalar2=SCH_B,
                            op0=Mult, op1=Add)
                    else:
                        nc.scalar.activation(out=pt[:, j, :], in_=st[:, :],
                                             func=Exp, scale=scale)
                return pt

            def pv(c, p, pt):
                h0, h1 = 2 * p, 2 * p + 1
                pv0 = acc_ps.tile([128, 512], FDT, tag="acc", name="pv0_t")
                pv1 = acc_ps.tile([128, 512], FDT, tag="acc", name="pv1_t")
                for j in range(nj):
                    nc.tensor.matmul(
                        pv0[0:VW, :], lhsT=v_sb[:, j, h0 * VW:(h0 + 1) * VW],
                        rhs=pt[:, j, 0:512],
                        start=(j == 0), stop=(j == nj - 1))
                    nc.tensor.matmul(
                        pv1[0:VW, :], lhsT=v_sb[:, j, h1 * VW:(h1 + 1) * VW],
                        rhs=pt[:, j, 512:1024],
                        start=(j == 0), stop=(j == nj - 1))
                return pv0, pv1

            def norm(c, p, pv0, pv1):
                # 1/l broadcast + attention-output normalization into aot
                if c == nqc - 1 and p == npr - 1:
                    # last pair: the DRAM-bounce latency would sit on the
                    # critical tail, so broadcast 1/l via K=1 PE matmuls
                    # (bf16 reciprocals - matmul dtypes must match).
                    ra = lr_pool.tile([1, 512], CDT, tag="ra16", name="ra_t")
                    rc = lr_pool.tile([1, 512], CDT, tag="rc16", name="rc_t")
                    with nc.allow_low_precision(reason="1/l bcast in bf16"):
                        nc.vector.reciprocal(out=ra[:, :],
                                             in_=pv0[DH:DH + 1, :])
                        nc.vector.reciprocal(out=rc[:, :],
                                             in_=pv1[DH:DH + 1, :])
                    rbp = acc_ps.tile([128, 512], FDT, tag="acc",
                                      name="rbp_t")
                    nc.tensor.matmul(rbp[0:64, :], lhsT=ones_sb[:, 0:64],
                                     rhs=ra[:, :], start=True, stop=True)
                    nc.tensor.matmul(rbp[64:128, :], lhsT=ones_sb[:, 0:64],
                                     rhs=rc[:, :], start=True, stop=True)
                    # tensor ops may read only one PSUM operand - stage the
                    # broadcast rows in SBUF via the (idle) Act engine
                    rbs = lr_pool.tile([128, 512], FDT, tag="rb", name="rbs_t")
                    nc.scalar.activation(out=rbs[:, :], in_=rbp[:, :],
                                         func=Copy, scale=1.0)
                    rb0, rb1 = rbs[0:64, :], rbs[64:128, :]
                else:
                    ra = lr_pool.tile([1, 512], FDT, tag="ra", name="ra_t")
                    rc = lr_pool.tile([1, 512], FDT, tag="rc", name="rc_t")
                    nc.vector.reciprocal(out=ra[:, :], in_=pv0[DH:DH + 1, :])
                    nc.vector.reciprocal(out=rc[:, :], in_=pv1[DH:DH + 1, :])
                    idx = (c * npr + p) * 2
                    nc.sync.dma_start(out=rsc_d.ap()[idx:idx + 1, :],
                                      in_=ra[:, :])
                    nc.sync.dma_start(out=rsc_d.ap()[idx + 1:idx + 2, :],
                                      in_=rc[:, :])
                    rb = lr_pool.tile([128, 512], FDT, tag="rb", name="rb_t")
                    src = rsc_d.ap()[idx:idx + 2, :].rearrange(
                        "r (o f) -> r o f", o=1).to_broadcast([2, DH, 512])
                    nc.gpsimd.dma_start(out=rb[:, :], in_=src)
                    rb0, rb1 = rb[0:64, :], rb[64:128, :]
                nc.vector.tensor_mul(
                    aot_sb[0:64, p, c * 512:(c + 1) * 512],
                    pv0[0:DH, :], rb0)
                nc.vector.tensor_mul(
                    aot_sb[64:128, p, c * 512:(c + 1) * 512],
                    pv1[0:DH, :], rb1)

            def outproj(c):
                for t in range(4 * c, 4 * c + 4):
                    f = acc_ps.tile([128, 512], FDT, tag="acc", name="f_t")
                    for s in range(inner // 128):
                        nc.tensor.matmul(
                            f[:, 0:dim],
                            lhsT=aot_sb[:, s, t * 128:(t + 1) * 128],
                            rhs=wout_sb[:, s, :],
                            start=(s == 0), stop=False)
                    nc.tensor.matmul(f[:, 0:dim], lhsT=ones_sb[:, :],
                                     rhs=bout_sb[:, :], start=False, stop=True)
                    fo = lr_pool.tile([128, dim], CDT, tag="fo", name="fo_t")
                    nc.scalar.activation(out=fo[:, :], in_=f[:, 0:dim],
                                         func=Copy, scale=1.0)
                    nc.sync.dma_start(
                        out=out_d.ap()[t * 128:(t + 1) * 128, :], in_=fo[:, :])

            pairs = [(c, p) for c in range(nqc) for p in range(npr)]
            prev = None
            for cp in pairs:
                pt = scores(*cp)
                if prev is not None:
                    (pc, pp), (ppv0, ppv1) = prev
                    norm(pc, pp, ppv0, ppv1)
                if oproj_early and prev is not None and prev[0][1] == npr - 1:
                    outproj(prev[0][0])
                pvs = pv(*cp, pt)
                if not oproj_early and prev is not None and prev[0][1] == npr - 1:
                    outproj(prev[0][0])
                prev = (cp, pvs)
            (pc, pp), (ppv0, ppv1) = prev
            norm(pc, pp, ppv0, ppv1)
            outproj(nqc - 1)
            attn_stack.close()

    if compile_module:
        nc.compile()
    return nc


def host_inputs(x, x_prev, w_q, w_kv, w_out, b_out, ncores=NCORES):
    bf16 = ml_dtypes.bfloat16
    b, n, dim = x.shape
    inner = w_q.shape[1]
    nq = (b * n) // ncores
    halves = ncores // b
    wq = np.ascontiguousarray(w_q).astype(bf16)
    wkv = np.ascontiguousarray(w_kv).astype(bf16)
    wout = np.ascontiguousarray(
        w_out.reshape(inner // 128, 128, dim).transpose(1, 0, 2)).astype(bf16)
    bout = np.ascontiguousarray(b_out).reshape(1, dim).astype(bf16)
    in_maps = []
    for c in range(ncores):
        bb, half = c // halves, c % halves
        xt = np.ascontiguousarray(
            x[bb, half * nq:(half + 1) * nq, :].T).astype(bf16)
        xpt = np.ascontiguousarray(x_prev[bb].T).astype(bf16)
        in_maps.append(dict(xt=xt, xpt=xpt, wq=wq, wkv=wkv, wout=wout,
                            bout=bout))
    return in_maps


def _get_module(**kw):
    key = tuple(sorted(kw.items()))
    if key not in _BUILT:
        _BUILT[key] = build_module(**kw)
    return _BUILT[key]


def kernel(x, x_prev, w_q, w_kv, w_out, b_out):
    from concourse.bass_utils import run_bass_kernel_spmd

    nc = _get_module()
    in_maps = host_inputs(x, x_prev, w_q, w_kv, w_out, b_out)
    res = run_bass_kernel_spmd(nc, in_maps, core_ids=list(range(NCORES)))

    nq = N // 2
    out = np.empty((B, N, DIM), np.float32)
    for c in range(NCORES):
        b, half = c // 2, c % 2
        out[b, half * nq:(half + 1) * nq, :] = \
            res.results[c]["out"].astype(np.float32)
    return out
